# revision 11
# baseline (speedup 1.0000x reference)
"""Gated dual-score (semantic+geometric/RoPE) causal attention layer on 8 TRN2 cores.

Sharding: data-parallel over batch (2) x tensor-parallel over heads (16 -> 4/core).
Core i: batch b = i // 4, heads hg = i % 4 -> heads [4*hg, 4*hg+4).
Each core computes a partial y (its heads' contribution, its batch); the host
sums the 4 partials per batch (the "all-reduce" of the row-sharded out-proj).

On-device layout trick: all projections consume xT (d on partitions, t free,
pre-transposed on host) and produce qT/kT in (d, t) layout. Scores are computed
transposed, (s on partitions, t free), which makes:
  - sem+geo fusion a single 128-contraction matmul (stacked [sem64|geo64] dims),
  - the gate fold into a pre-scale of qT columns,
  - P@V consume the exp tile directly with V in natural (t, dv) layout,
  - the softmax denominator a ones-vector matmul.
Softmax skips max-subtraction (scores are O(5) by construction; fp32/bf16 safe).
Compute dtype bf16 (fp32 matmul costs 2x cycles on TRN2), fp32 accumulation.
"""

import sys
from contextlib import ExitStack

import numpy as np

sys.path.insert(0, "/opt/trn_rl_repo")

import ml_dtypes  # noqa: E402

import concourse.bass as bass  # noqa: E402
from concourse import bacc  # noqa: E402
import concourse.mybir as mybir  # noqa: E402
import concourse.tile as tile  # noqa: E402
from concourse.bass_utils import run_bass_kernel_spmd  # noqa: E402

B, T, D, H = 2, 2048, 2048, 16
SEM_HD = GEO_HD = 64
V_HD = 128
HL = 4  # heads per core
CL = HL * V_HD  # local v-dim (512)
ROPE_BASE = 10000.0
NEG_INF = -1e9

KT = D // 128  # 16 k-tiles over the contraction dim
TT = T // 128  # 16 token tiles of 128
TC = T // 512  # 4 token chunks of 512
BF = mybir.dt.bfloat16
F32 = mybir.dt.float32
NPBF = ml_dtypes.bfloat16

_CACHED_NC = None


def _build_nc():
    nc = bacc.Bacc()

    xt_d = nc.declare_dram_parameter("xt", [KT, 128, T], BF, isOutput=False)
    wq_d = nc.declare_dram_parameter("wq", [HL, 128, KT, 128], BF, isOutput=False)
    wk_d = nc.declare_dram_parameter("wk", [HL, 128, KT, 128], BF, isOutput=False)
    wv_d = nc.declare_dram_parameter("wv", [KT, 128, CL], BF, isOutput=False)
    wo_d = nc.declare_dram_parameter("wo", [HL, 128, D], BF, isOutput=False)
    wg_d = nc.declare_dram_parameter("wg", [128, KT, 2 * HL], BF, isOutput=False)
    glog_d = nc.declare_dram_parameter("glog", [2 * HL, 1], F32, isOutput=False)
    crep_d = nc.declare_dram_parameter("crep", [128, T], BF, isOutput=False)
    srep_d = nc.declare_dram_parameter("srep", [128, T], BF, isOutput=False)
    maskd_d = nc.declare_dram_parameter("maskd", [128, 128], F32, isOutput=False)
    selc_d = nc.declare_dram_parameter("selc", [2 * HL, HL * 128], BF, isOutput=False)
    gsv_d = nc.declare_dram_parameter("gsv", [2 * HL, 1], F32, isOutput=False)
    gbv_d = nc.declare_dram_parameter("gbv", [2 * HL, 1], F32, isOutput=False)
    y_d = nc.declare_dram_parameter("y", [T, D], F32, isOutput=True)

    with tile.TileContext(nc) as tc, ExitStack() as ctx:
        singles = ctx.enter_context(tc.tile_pool(name="singles", bufs=1))
        xpool = ctx.enter_context(tc.tile_pool(name="xpool", bufs=1))
        vpool = ctx.enter_context(tc.tile_pool(name="vpool", bufs=1))
        wqk_pool = ctx.enter_context(tc.tile_pool(name="wqk", bufs=2))
        qk_pool = ctx.enter_context(tc.tile_pool(name="qk", bufs=2))
        ot_pool = ctx.enter_context(tc.tile_pool(name="ot", bufs=1))
        wo_pool = ctx.enter_context(tc.tile_pool(name="wo", bufs=1))
        p_pool = ctx.enter_context(tc.tile_pool(name="pp", bufs=6))
        sc_pool = ctx.enter_context(tc.tile_pool(name="scratch", bufs=2))
        y_pool = ctx.enter_context(tc.tile_pool(name="ysb", bufs=2))

        ps_a = ctx.enter_context(tc.tile_pool(name="ps_a", bufs=3, space="PSUM"))
        ps_s = ctx.enter_context(tc.tile_pool(name="ps_s", bufs=2, space="PSUM"))
        ps_o = ctx.enter_context(tc.tile_pool(name="ps_o", bufs=1, space="PSUM"))
        ps_d = ctx.enter_context(tc.tile_pool(name="ps_d", bufs=1, space="PSUM"))
        ps_g = ctx.enter_context(tc.tile_pool(name="ps_g", bufs=1, space="PSUM"))

        # ---- static tables / constants ----
        crep = singles.tile([128, T], BF)
        srep = singles.tile([128, T], BF)
        maskd = singles.tile([128, 128], F32)
        glog = singles.tile([2 * HL, 1], F32)
        nc.sync.dma_start(out=crep, in_=crep_d[:])
        nc.sync.dma_start(out=srep, in_=srep_d[:])
        nc.sync.dma_start(out=maskd, in_=maskd_d[:])
        nc.sync.dma_start(out=glog, in_=glog_d[:])

        ones_col = singles.tile([128, 1], BF)  # denom lhsT
        nc.vector.memset(ones_col, 1.0)
        one_row = singles.tile([1, 128], F32)  # recip-bcast lhsT
        nc.vector.memset(one_row, 1.0)
        # per-head gate-broadcast selector: rows 0:4 pick g/8 into psum rows
        # 0:64, rows 4:8 pick (1-g)/8 into psum rows 64:128 (host-built)
        selc = singles.tile([2 * HL, HL * 128], BF)
        nc.sync.dma_start(out=selc, in_=selc_d[:])
        gsv = singles.tile([2 * HL, 1], F32)  # [1/8 x4; -1/8 x4]
        gbv = singles.tile([2 * HL, 1], F32)  # [0 x4; 1/8 x4]
        nc.sync.dma_start(out=gsv, in_=gsv_d[:])
        nc.sync.dma_start(out=gbv, in_=gbv_d[:])

        # ---- x^T resident in SBUF ----
        xt = xpool.tile([128, KT, T], BF)
        for k in range(KT):
            nc.sync.dma_start(out=xt[:, k, :], in_=xt_d[k])

        wg = singles.tile([128, KT, 2 * HL], BF)
        nc.sync.dma_start(out=wg, in_=wg_d[:])

        # ---- gate projection: rows 0:4 and 4:8 both sigmoid(glog + x@gate_w);
        # gcomb rows 0:4 = g/8, rows 4:8 = (1-g)/8, fp32 (8, T)
        gcomb = singles.tile([2 * HL, T], BF)
        for j in range(TC):
            tsl = slice(512 * j, 512 * (j + 1))
            pg = ps_g.tile([2 * HL, 512], F32, tag="gsmall", name="pg")
            for k in range(KT):
                nc.tensor.matmul(
                    pg, wg[:, k, :], xt[:, k, tsl], start=(k == 0), stop=(k == KT - 1)
                )
            gsig = sc_pool.tile([2 * HL, 512], F32, tag="gsig", bufs=1)
            nc.scalar.activation(
                gsig, pg, mybir.ActivationFunctionType.Sigmoid, bias=glog
            )
            nc.scalar.activation(
                gcomb[:, tsl],
                gsig,
                mybir.ActivationFunctionType.Identity,
                scale=gsv,
                bias=gbv,
            )

        # ---- V projection, natural (t, dv) layout: lhsT = xT tile, rhs = w_v ----
        v_sb = vpool.tile([128, TT, CL], BF)
        with tc.tile_pool(name="wvpool", bufs=1) as wv_pool:
            wv = wv_pool.tile([128, KT, CL], BF)
            for k in range(KT):
                nc.sync.dma_start(out=wv[:, k, :], in_=wv_d[k])
            for i in range(TT):
                pv = ps_a.tile([128, CL], F32, tag="big", name="pv")
                for k in range(KT):
                    nc.tensor.matmul(
                        pv,
                        xt[:, k, 128 * i : 128 * (i + 1)],
                        wv[:, k, :],
                        start=(k == 0),
                        stop=(k == KT - 1),
                    )
                nc.scalar.copy(v_sb[:, i, :], pv)

        # ---- per-head: QK projection (+gate/RoPE fusion) then attention ----
        outT = [
            ot_pool.tile([128, T], BF, tag=f"ot{h}", name=f"ot{h}") for h in range(HL)
        ]
        for h in range(HL):
            wq_sb = wqk_pool.tile([128, KT, 128], BF, tag="wq")
            wk_sb = wqk_pool.tile([128, KT, 128], BF, tag="wk")
            nc.sync.dma_start(out=wq_sb, in_=wq_d[h])
            nc.sync.dma_start(out=wk_sb, in_=wk_d[h])
            qstk = qk_pool.tile([128, T], BF, tag="qstk")
            kstk = qk_pool.tile([128, T], BF, tag="kstk")

            for j in range(TC):
                tsl = slice(512 * j, 512 * (j + 1))
                pq = ps_a.tile([128, 512], F32, tag="big", name="pq")
                pk = ps_a.tile([128, 512], F32, tag="big", name="pk")
                for k in range(KT):
                    nc.tensor.matmul(
                        pq, wq_sb[:, k, :], xt[:, k, tsl],
                        start=(k == 0), stop=(k == KT - 1),
                    )
                for k in range(KT):
                    nc.tensor.matmul(
                        pk, wk_sb[:, k, :], xt[:, k, tsl],
                        start=(k == 0), stop=(k == KT - 1),
                    )
                # gate broadcast: rows 0:64 <- g/8, rows 64:128 <- (1-g)/8
                gbb = ps_g.tile([128, 512], F32, tag="gsmall", name="gbb")
                nc.tensor.matmul(
                    gbb,
                    selc[:, 128 * h : 128 * (h + 1)],
                    gcomb[:, tsl],
                    start=True,
                    stop=True,
                )

                gbs = sc_pool.tile([128, 512], F32, tag="gbs", bufs=1)
                nc.scalar.copy(gbs, gbb)
                # q: sem rows scaled by g/8
                nc.vector.tensor_mul(qstk[0:64, tsl], pq[0:64, :], gbs[0:64, :])
                # q geo: rotate by RoPE then scale by (1-g)/8
                m1 = sc_pool.tile([128, 512], F32, tag="m1")
                m2 = sc_pool.tile([128, 512], F32, tag="m2")
                m2b = sc_pool.tile([128, 512], F32, tag="m2")
                nc.vector.tensor_mul(m1[64:128, :], pq[64:128, :], crep[64:128, tsl])
                nc.vector.tensor_mul(m2[64:128, :], pq[64:128, :], srep[64:128, tsl])
                nc.vector.tensor_copy(m2b[64:96, :], m2[96:128, :])
                nc.vector.tensor_copy(m2b[96:128, :], m2[64:96, :])
                nc.vector.tensor_sub(m1[64:96, :], m1[64:96, :], m2b[64:96, :])
                nc.vector.tensor_add(m1[96:128, :], m1[96:128, :], m2b[96:128, :])
                nc.vector.tensor_mul(qstk[64:128, tsl], m1[64:128, :], gbs[64:128, :])
                # k: sem rows copied, geo rows rotated (scale folded into q side)
                nc.scalar.copy(kstk[0:64, tsl], pk[0:64, :])
                km1 = sc_pool.tile([128, 512], F32, tag="m1")
                km2 = sc_pool.tile([128, 512], F32, tag="m2")
                km2b = sc_pool.tile([128, 512], F32, tag="m2")
                nc.vector.tensor_mul(km1[64:128, :], pk[64:128, :], crep[64:128, tsl])
                nc.vector.tensor_mul(km2[64:128, :], pk[64:128, :], srep[64:128, tsl])
                nc.vector.tensor_copy(km2b[64:96, :], km2[96:128, :])
                nc.vector.tensor_copy(km2b[96:128, :], km2[64:96, :])
                nc.vector.tensor_sub(kstk[64:96, tsl], km1[64:96, :], km2b[64:96, :])
                nc.vector.tensor_add(kstk[96:128, tsl], km1[96:128, :], km2b[96:128, :])

            # attention for this head, chunk by chunk
            for j in range(TC):
                tsl = slice(512 * j, 512 * (j + 1))
                po = ps_o.tile([128, 512], F32, tag="po")
                pd = ps_d.tile([1, 512], F32, tag="pd")
                n_s = 4 * (j + 1)
                for s in range(n_s):
                    dj = s - 4 * j  # >=0 on diagonal tiles
                    c0 = 128 * dj if dj >= 0 else 0
                    ssl = slice(128 * s, 128 * (s + 1))
                    ps = ps_s.tile([128, 512], F32, tag="ps", name="ps")
                    nc.tensor.matmul(
                        ps[:, c0:512],
                        kstk[:, ssl],
                        qstk[:, 512 * j + c0 : 512 * (j + 1)],
                        start=True,
                        stop=True,
                    )
                    if dj >= 0:
                        nc.vector.tensor_add(
                            ps[:, c0 : c0 + 128], ps[:, c0 : c0 + 128], maskd
                        )
                    pt = p_pool.tile([128, 512], BF, tag="pt", name="pt")
                    nc.scalar.activation(
                        pt[:, c0:512], ps[:, c0:512], mybir.ActivationFunctionType.Exp
                    )
                    nc.tensor.matmul(
                        po[:, c0:512],
                        v_sb[:, s, 128 * h : 128 * (h + 1)],
                        pt[:, c0:512],
                        start=(s == 0),
                        stop=(s == n_s - 1),
                    )
                    nc.tensor.matmul(
                        pd[:, c0:512],
                        ones_col,
                        pt[:, c0:512],
                        start=(s == 0),
                        stop=(s == n_s - 1),
                    )
                # normalize: outT = po * (1/denom) broadcast down partitions
                rec = sc_pool.tile([1, 512], F32, tag="rec")
                nc.vector.reciprocal(rec, pd)
                rbc = ps_g.tile([128, 512], F32, tag="gsmall", name="rbc")
                nc.tensor.matmul(rbc, one_row, rec, start=True, stop=True)
                rbs = sc_pool.tile([128, 512], F32, tag="rbs")
                nc.scalar.copy(rbs, rbc)
                nc.vector.tensor_mul(outT[h][:, tsl], po, rbs)

        # ---- out-projection: y[t, e] = sum_h outT_h^T @ wo_h ----
        wo_sb = [
            wo_pool.tile([128, D], BF, tag=f"wo{h}", name=f"wo{h}") for h in range(HL)
        ]
        for h in range(HL):
            nc.sync.dma_start(out=wo_sb[h], in_=wo_d[h])
        for i in range(TT):
            for ec in range(D // 512):
                py = ps_a.tile([128, 512], F32, tag="big", name="py")
                for h in range(HL):
                    nc.tensor.matmul(
                        py,
                        outT[h][:, 128 * i : 128 * (i + 1)],
                        wo_sb[h][:, 512 * ec : 512 * (ec + 1)],
                        start=(h == 0),
                        stop=(h == HL - 1),
                    )
                ysb = y_pool.tile([128, 512], F32, tag="ysb")
                nc.vector.tensor_copy(ysb, py)
                nc.sync.dma_start(
                    out=y_d[128 * i : 128 * (i + 1), 512 * ec : 512 * (ec + 1)],
                    in_=ysb,
                )

    nc.finalize()
    return nc


def _host_prep(x, w_q_sem, w_k_sem, w_q_geo, w_k_geo, w_v, w_out, gate_logit, gate_w):
    """Build the 8 per-core input maps (all numpy, bf16 where matmul-bound)."""
    half = GEO_HD // 2  # 32
    inv_freq = 1.0 / (ROPE_BASE ** (np.arange(half, dtype=np.float64) / half))
    pos = np.arange(T, dtype=np.float64)
    ang = pos[None, :] * inv_freq[:, None]  # (32, T)
    crep = np.zeros((128, T), dtype=NPBF)
    srep = np.zeros((128, T), dtype=NPBF)
    crep[64:96] = np.cos(ang)
    crep[96:128] = np.cos(ang)
    srep[64:96] = np.sin(ang)
    srep[96:128] = np.sin(ang)

    p_i = np.arange(128)
    maskd = np.where(p_i[:, None] <= p_i[None, :], 0.0, NEG_INF).astype(np.float32)

    # per-head stacked [sem64 | geo64] projection weights, (128, KT, 128) layout
    def stack_heads(wsem, wgeo):
        out = []
        for h in range(H):
            blk = np.concatenate(
                [wsem[:, 64 * h : 64 * (h + 1)], wgeo[:, 64 * h : 64 * (h + 1)]],
                axis=1,
            )  # (D, 128)
            out.append(
                np.ascontiguousarray(
                    blk.reshape(KT, 128, 128).transpose(1, 0, 2)
                ).astype(NPBF)
            )
        return out  # H x (128, KT, 128)

    wq_all = stack_heads(w_q_sem, w_q_geo)
    wk_all = stack_heads(w_k_sem, w_k_geo)

    in_maps = []
    for core in range(8):
        b, hg = core // 4, core % 4
        heads = range(4 * hg, 4 * hg + 4)
        xt = np.ascontiguousarray(x[b].T).astype(NPBF).reshape(KT, 128, T)
        wq = np.stack([wq_all[h] for h in heads])
        wk = np.stack([wk_all[h] for h in heads])
        wv = w_v[:, CL * hg : CL * (hg + 1)].reshape(KT, 128, CL).astype(NPBF)
        wo = w_out[CL * hg : CL * (hg + 1), :].reshape(HL, 128, D).astype(NPBF)
        gwl = gate_w[:, 4 * hg : 4 * hg + 4]  # (D, 4)
        gw2 = np.concatenate([gwl, gwl], axis=1)  # (D, 8) duplicated
        wg = np.ascontiguousarray(
            gw2.reshape(KT, 128, 2 * HL).transpose(1, 0, 2)
        ).astype(NPBF)
        selc = np.zeros((2 * HL, HL * 128), dtype=NPBF)
        for h in range(HL):
            selc[h, 128 * h : 128 * h + 64] = 1.0
            selc[HL + h, 128 * h + 64 : 128 * h + 128] = 1.0
        gsv = np.array([0.125] * HL + [-0.125] * HL, dtype=np.float32).reshape(2 * HL, 1)
        gbv = np.array([0.0] * HL + [0.125] * HL, dtype=np.float32).reshape(2 * HL, 1)
        gll = gate_logit[4 * hg : 4 * hg + 4]
        glog = np.ascontiguousarray(
            np.concatenate([gll, gll]).reshape(2 * HL, 1)
        ).astype(np.float32)
        in_maps.append(
            {
                "xt": xt,
                "wq": wq,
                "wk": wk,
                "wv": np.ascontiguousarray(wv),
                "wo": np.ascontiguousarray(wo),
                "wg": wg,
                "glog": glog,
                "crep": crep,
                "srep": srep,
                "maskd": maskd,
                "selc": selc,
                "gsv": gsv,
                "gbv": gbv,
            }
        )
    return in_maps


def _run(inputs, trace=False):
    global _CACHED_NC
    if _CACHED_NC is None:
        _CACHED_NC = _build_nc()
    in_maps = _host_prep(**{k: np.asarray(v) for k, v in inputs.items()})
    res = run_bass_kernel_spmd(
        _CACHED_NC, in_maps, core_ids=list(range(8)), trace=trace
    )
    y = np.zeros((B, T, D), dtype=np.float32)
    for core in range(8):
        y[core // 4] += res.results[core]["y"]
    return y, res


def kernel(**inputs) -> np.ndarray:
    y, _ = _run(inputs, trace=False)
    return y


# revision 12
# speedup vs baseline: 1.1503x; 1.1503x over previous
"""Gated dual-score (semantic+geometric/RoPE) causal attention layer on 8 TRN2 cores.

Sharding: data-parallel over batch (2) x tensor-parallel over heads (16 -> 4/core).
Core i: batch b = i // 4, heads hg = i % 4 -> heads [4*hg, 4*hg+4).
Each core computes a partial y (its heads' contribution, its batch); the host
sums the 4 partials per batch (the "all-reduce" of the row-sharded out-proj).

On-device layout trick: all projections consume xT (d on partitions, t free,
pre-transposed on host) and produce qT/kT in (d, t) layout. Scores are computed
transposed, (s on partitions, t free), which makes:
  - sem+geo fusion a single 128-contraction matmul (stacked [sem64|geo64] dims),
  - the gate fold into a pre-scale of qT columns,
  - P@V consume the exp tile directly with V in natural (t, dv) layout,
  - the softmax denominator a ones-vector matmul.
Softmax skips max-subtraction (scores are O(5) by construction; fp32/bf16 safe).
Compute dtype bf16 (fp32 matmul costs 2x cycles on TRN2), fp32 accumulation.
"""

import sys
from contextlib import ExitStack

import numpy as np

sys.path.insert(0, "/opt/trn_rl_repo")

import ml_dtypes  # noqa: E402

import concourse.bass as bass  # noqa: E402
from concourse import bacc  # noqa: E402
import concourse.mybir as mybir  # noqa: E402
import concourse.tile as tile  # noqa: E402
from concourse.bass_utils import run_bass_kernel_spmd  # noqa: E402

B, T, D, H = 2, 2048, 2048, 16
SEM_HD = GEO_HD = 64
V_HD = 128
HL = 4  # heads per core
CL = HL * V_HD  # local v-dim (512)
ROPE_BASE = 10000.0
NEG_INF = -1e9

KT = D // 128  # 16 k-tiles over the contraction dim
TT = T // 128  # 16 token tiles of 128
TC = T // 512  # 4 token chunks of 512
BF = mybir.dt.bfloat16
F32 = mybir.dt.float32
NPBF = ml_dtypes.bfloat16

_CACHED_NC = None


def _build_nc():
    nc = bacc.Bacc()

    xt_d = nc.declare_dram_parameter("xt", [KT, 128, T], BF, isOutput=False)
    wq_d = nc.declare_dram_parameter("wq", [HL, 128, KT, 128], BF, isOutput=False)
    wk_d = nc.declare_dram_parameter("wk", [HL, 128, KT, 128], BF, isOutput=False)
    wv_d = nc.declare_dram_parameter("wv", [KT, 128, CL], BF, isOutput=False)
    wo_d = nc.declare_dram_parameter("wo", [HL, 128, D], BF, isOutput=False)
    wg_d = nc.declare_dram_parameter("wg", [128, KT, 2 * HL], BF, isOutput=False)
    glog_d = nc.declare_dram_parameter("glog", [2 * HL, 1], F32, isOutput=False)
    crep_d = nc.declare_dram_parameter("crep", [128, T], BF, isOutput=False)
    srep_d = nc.declare_dram_parameter("srep", [128, T], BF, isOutput=False)
    maskd_d = nc.declare_dram_parameter("maskd", [128, 128], F32, isOutput=False)
    selc_d = nc.declare_dram_parameter("selc", [2 * HL, HL * 128], BF, isOutput=False)
    gsv_d = nc.declare_dram_parameter("gsv", [2 * HL, 1], F32, isOutput=False)
    gbv_d = nc.declare_dram_parameter("gbv", [2 * HL, 1], F32, isOutput=False)
    y_d = nc.declare_dram_parameter("y", [T, D], F32, isOutput=True)

    with tile.TileContext(nc) as tc, ExitStack() as ctx:
        singles = ctx.enter_context(tc.tile_pool(name="singles", bufs=1))
        xpool = ctx.enter_context(tc.tile_pool(name="xpool", bufs=1))
        vpool = ctx.enter_context(tc.tile_pool(name="vpool", bufs=1))
        wqk_pool = ctx.enter_context(tc.tile_pool(name="wqk", bufs=2))
        qk_pool = ctx.enter_context(tc.tile_pool(name="qk", bufs=2))
        ot_pool = ctx.enter_context(tc.tile_pool(name="ot", bufs=1))
        wo_pool = ctx.enter_context(tc.tile_pool(name="wo", bufs=1))
        p_pool = ctx.enter_context(tc.tile_pool(name="pp", bufs=6))
        sc_pool = ctx.enter_context(tc.tile_pool(name="scratch", bufs=2))
        y_pool = ctx.enter_context(tc.tile_pool(name="ysb", bufs=2))

        ps_a = ctx.enter_context(tc.tile_pool(name="ps_a", bufs=3, space="PSUM"))
        ps_s = ctx.enter_context(tc.tile_pool(name="ps_s", bufs=2, space="PSUM"))
        ps_o = ctx.enter_context(tc.tile_pool(name="ps_o", bufs=1, space="PSUM"))
        ps_d = ctx.enter_context(tc.tile_pool(name="ps_d", bufs=1, space="PSUM"))
        ps_g = ctx.enter_context(tc.tile_pool(name="ps_g", bufs=1, space="PSUM"))

        # ---- static tables / constants ----
        crep = singles.tile([128, T], BF)
        srep = singles.tile([128, T], BF)
        maskd = singles.tile([128, 128], F32)
        glog = singles.tile([2 * HL, 1], F32)
        nc.sync.dma_start(out=crep, in_=crep_d[:])
        nc.sync.dma_start(out=srep, in_=srep_d[:])
        nc.sync.dma_start(out=maskd, in_=maskd_d[:])
        nc.sync.dma_start(out=glog, in_=glog_d[:])

        ones_col = singles.tile([128, 1], BF)  # denom lhsT
        nc.vector.memset(ones_col, 1.0)
        one_row = singles.tile([1, 128], BF)  # denom-bcast lhsT
        nc.vector.memset(one_row, 1.0)
        # per-head gate-broadcast selector: rows 0:4 pick g/8 into psum rows
        # 0:64, rows 4:8 pick (1-g)/8 into psum rows 64:128 (host-built)
        selc = singles.tile([2 * HL, HL * 128], BF)
        nc.sync.dma_start(out=selc, in_=selc_d[:])
        gsv = singles.tile([2 * HL, 1], F32)  # [1/8 x4; -1/8 x4]
        gbv = singles.tile([2 * HL, 1], F32)  # [0 x4; 1/8 x4]
        nc.sync.dma_start(out=gsv, in_=gsv_d[:])
        nc.sync.dma_start(out=gbv, in_=gbv_d[:])

        # ---- x^T resident in SBUF ----
        xt = xpool.tile([128, KT, T], BF)
        for k in range(KT):
            nc.sync.dma_start(out=xt[:, k, :], in_=xt_d[k])

        wg = singles.tile([128, KT, 2 * HL], BF)
        nc.sync.dma_start(out=wg, in_=wg_d[:])

        # ---- gate projection: rows 0:4 and 4:8 both sigmoid(glog + x@gate_w);
        # gcomb rows 0:4 = g/8, rows 4:8 = (1-g)/8, fp32 (8, T)
        gcomb = singles.tile([2 * HL, T], BF)
        for j in range(TC):
            tsl = slice(512 * j, 512 * (j + 1))
            pg = ps_g.tile([2 * HL, 512], F32, tag="gsmall", name="pg")
            for k in range(KT):
                nc.tensor.matmul(
                    pg, wg[:, k, :], xt[:, k, tsl], start=(k == 0), stop=(k == KT - 1)
                )
            gsig = sc_pool.tile([2 * HL, 512], F32, tag="gsig", bufs=1)
            nc.scalar.activation(
                gsig, pg, mybir.ActivationFunctionType.Sigmoid, bias=glog
            )
            nc.scalar.activation(
                gcomb[:, tsl],
                gsig,
                mybir.ActivationFunctionType.Identity,
                scale=gsv,
                bias=gbv,
            )

        # ---- V projection, natural (t, dv) layout: lhsT = xT tile, rhs = w_v ----
        v_sb = vpool.tile([128, TT, CL], BF)
        with tc.tile_pool(name="wvpool", bufs=1) as wv_pool:
            wv = wv_pool.tile([128, KT, CL], BF)
            for k in range(KT):
                nc.sync.dma_start(out=wv[:, k, :], in_=wv_d[k])
            for i in range(TT):
                pv = ps_a.tile([128, CL], F32, tag="big", name="pv")
                for k in range(KT):
                    nc.tensor.matmul(
                        pv,
                        xt[:, k, 128 * i : 128 * (i + 1)],
                        wv[:, k, :],
                        start=(k == 0),
                        stop=(k == KT - 1),
                    )
                nc.scalar.copy(v_sb[:, i, :], pv)

        # ---- per-head: QK projection (+gate/RoPE fusion) then attention ----
        outT = [
            ot_pool.tile([128, T], BF, tag=f"ot{h}", name=f"ot{h}") for h in range(HL)
        ]
        for h in range(HL):
            wq_sb = wqk_pool.tile([128, KT, 128], BF, tag="wq")
            wk_sb = wqk_pool.tile([128, KT, 128], BF, tag="wk")
            nc.sync.dma_start(out=wq_sb, in_=wq_d[h])
            nc.sync.dma_start(out=wk_sb, in_=wk_d[h])
            qstk = qk_pool.tile([128, T], BF, tag="qstk")
            kstk = qk_pool.tile([128, T], BF, tag="kstk")

            for j in range(TC):
                tsl = slice(512 * j, 512 * (j + 1))
                pq = ps_a.tile([128, 512], F32, tag="big", name="pq")
                pk = ps_a.tile([128, 512], F32, tag="big", name="pk")
                for k in range(KT):
                    nc.tensor.matmul(
                        pq, wq_sb[:, k, :], xt[:, k, tsl],
                        start=(k == 0), stop=(k == KT - 1),
                    )
                for k in range(KT):
                    nc.tensor.matmul(
                        pk, wk_sb[:, k, :], xt[:, k, tsl],
                        start=(k == 0), stop=(k == KT - 1),
                    )
                # gate broadcast: rows 0:64 <- g/8, rows 64:128 <- (1-g)/8
                gbb = ps_g.tile([128, 512], F32, tag="gsmall", name="gbb")
                nc.tensor.matmul(
                    gbb,
                    selc[:, 128 * h : 128 * (h + 1)],
                    gcomb[:, tsl],
                    start=True,
                    stop=True,
                )

                gbs = sc_pool.tile([128, 512], F32, tag="gbs", bufs=1)
                nc.scalar.copy(gbs, gbb)
                # q: sem rows scaled by g/8
                nc.vector.tensor_mul(qstk[0:64, tsl], pq[0:64, :], gbs[0:64, :])
                # q geo: rotate by RoPE then scale by (1-g)/8
                m1 = sc_pool.tile([128, 512], F32, tag="m1")
                m2 = sc_pool.tile([128, 512], F32, tag="m2")
                m2b = sc_pool.tile([128, 512], F32, tag="m2")
                nc.vector.tensor_mul(m1[64:128, :], pq[64:128, :], crep[64:128, tsl])
                nc.vector.tensor_mul(m2[64:128, :], pq[64:128, :], srep[64:128, tsl])
                nc.vector.tensor_copy(m2b[64:96, :], m2[96:128, :])
                nc.vector.tensor_copy(m2b[96:128, :], m2[64:96, :])
                nc.vector.tensor_sub(m1[64:96, :], m1[64:96, :], m2b[64:96, :])
                nc.vector.tensor_add(m1[96:128, :], m1[96:128, :], m2b[96:128, :])
                nc.vector.tensor_mul(qstk[64:128, tsl], m1[64:128, :], gbs[64:128, :])
                # k: sem rows copied, geo rows rotated (scale folded into q side)
                nc.scalar.copy(kstk[0:64, tsl], pk[0:64, :])
                km1 = sc_pool.tile([128, 512], F32, tag="m1")
                km2 = sc_pool.tile([128, 512], F32, tag="m2")
                km2b = sc_pool.tile([128, 512], F32, tag="m2")
                nc.vector.tensor_mul(km1[64:128, :], pk[64:128, :], crep[64:128, tsl])
                nc.vector.tensor_mul(km2[64:128, :], pk[64:128, :], srep[64:128, tsl])
                nc.vector.tensor_copy(km2b[64:96, :], km2[96:128, :])
                nc.vector.tensor_copy(km2b[96:128, :], km2[64:96, :])
                nc.vector.tensor_sub(kstk[64:96, tsl], km1[64:96, :], km2b[64:96, :])
                nc.vector.tensor_add(kstk[96:128, tsl], km1[96:128, :], km2b[96:128, :])

            # attention for this head, chunk by chunk
            for j in range(TC):
                tsl = slice(512 * j, 512 * (j + 1))
                po = ps_o.tile([128, 512], F32, tag="po")
                pd = ps_d.tile([1, 512], F32, tag="pd")
                n_s = 4 * (j + 1)
                for s in range(n_s):
                    dj = s - 4 * j  # >=0 on diagonal tiles
                    c0 = 128 * dj if dj >= 0 else 0
                    ssl = slice(128 * s, 128 * (s + 1))
                    ps = ps_s.tile([128, 512], F32, tag="ps", name="ps")
                    nc.tensor.matmul(
                        ps[:, c0:512],
                        kstk[:, ssl],
                        qstk[:, 512 * j + c0 : 512 * (j + 1)],
                        start=True,
                        stop=True,
                    )
                    if dj >= 0:
                        nc.vector.tensor_add(
                            ps[:, c0 : c0 + 128], ps[:, c0 : c0 + 128], maskd
                        )
                    pt = p_pool.tile([128, 512], BF, tag="pt", name="pt")
                    nc.scalar.activation(
                        pt[:, c0:512], ps[:, c0:512], mybir.ActivationFunctionType.Exp
                    )
                    nc.tensor.matmul(
                        po[:, c0:512],
                        v_sb[:, s, 128 * h : 128 * (h + 1)],
                        pt[:, c0:512],
                        start=(s == 0),
                        stop=(s == n_s - 1),
                    )
                    nc.tensor.matmul(
                        pd[:, c0:512],
                        ones_col,
                        pt[:, c0:512],
                        start=(s == 0),
                        stop=(s == n_s - 1),
                    )
                # normalize: broadcast denom down partitions (bf16 matmul),
                # then approx-reciprocal the full tile on DVE
                pdb = sc_pool.tile([1, 512], BF, tag="rec")
                nc.scalar.copy(pdb, pd)
                rbc = ps_g.tile([128, 512], F32, tag="gsmall", name="rbc")
                nc.tensor.matmul(rbc, one_row, pdb, start=True, stop=True)
                rbs = sc_pool.tile([128, 512], F32, tag="rbs")
                nc.vector.reciprocal_approx_fast(out=rbs, in_=rbc)
                nc.vector.tensor_mul(outT[h][:, tsl], po, rbs)

        # ---- out-projection: y[t, e] = sum_h outT_h^T @ wo_h ----
        wo_sb = [
            wo_pool.tile([128, D], BF, tag=f"wo{h}", name=f"wo{h}") for h in range(HL)
        ]
        for h in range(HL):
            nc.sync.dma_start(out=wo_sb[h], in_=wo_d[h])
        for i in range(TT):
            for ec in range(D // 512):
                py = ps_a.tile([128, 512], F32, tag="big", name="py")
                for h in range(HL):
                    nc.tensor.matmul(
                        py,
                        outT[h][:, 128 * i : 128 * (i + 1)],
                        wo_sb[h][:, 512 * ec : 512 * (ec + 1)],
                        start=(h == 0),
                        stop=(h == HL - 1),
                    )
                ysb = y_pool.tile([128, 512], F32, tag="ysb")
                nc.scalar.copy(ysb, py)
                nc.sync.dma_start(
                    out=y_d[128 * i : 128 * (i + 1), 512 * ec : 512 * (ec + 1)],
                    in_=ysb,
                )

    nc.finalize()
    return nc


def _host_prep(x, w_q_sem, w_k_sem, w_q_geo, w_k_geo, w_v, w_out, gate_logit, gate_w):
    """Build the 8 per-core input maps (all numpy, bf16 where matmul-bound)."""
    half = GEO_HD // 2  # 32
    inv_freq = 1.0 / (ROPE_BASE ** (np.arange(half, dtype=np.float64) / half))
    pos = np.arange(T, dtype=np.float64)
    ang = pos[None, :] * inv_freq[:, None]  # (32, T)
    crep = np.zeros((128, T), dtype=NPBF)
    srep = np.zeros((128, T), dtype=NPBF)
    crep[64:96] = np.cos(ang)
    crep[96:128] = np.cos(ang)
    srep[64:96] = np.sin(ang)
    srep[96:128] = np.sin(ang)

    p_i = np.arange(128)
    maskd = np.where(p_i[:, None] <= p_i[None, :], 0.0, NEG_INF).astype(np.float32)

    # per-head stacked [sem64 | geo64] projection weights, (128, KT, 128) layout
    def stack_heads(wsem, wgeo):
        out = []
        for h in range(H):
            blk = np.concatenate(
                [wsem[:, 64 * h : 64 * (h + 1)], wgeo[:, 64 * h : 64 * (h + 1)]],
                axis=1,
            )  # (D, 128)
            out.append(
                np.ascontiguousarray(
                    blk.reshape(KT, 128, 128).transpose(1, 0, 2)
                ).astype(NPBF)
            )
        return out  # H x (128, KT, 128)

    wq_all = stack_heads(w_q_sem, w_q_geo)
    wk_all = stack_heads(w_k_sem, w_k_geo)

    in_maps = []
    for core in range(8):
        b, hg = core // 4, core % 4
        heads = range(4 * hg, 4 * hg + 4)
        xt = np.ascontiguousarray(x[b].T).astype(NPBF).reshape(KT, 128, T)
        wq = np.stack([wq_all[h] for h in heads])
        wk = np.stack([wk_all[h] for h in heads])
        wv = w_v[:, CL * hg : CL * (hg + 1)].reshape(KT, 128, CL).astype(NPBF)
        wo = w_out[CL * hg : CL * (hg + 1), :].reshape(HL, 128, D).astype(NPBF)
        gwl = gate_w[:, 4 * hg : 4 * hg + 4]  # (D, 4)
        gw2 = np.concatenate([gwl, gwl], axis=1)  # (D, 8) duplicated
        wg = np.ascontiguousarray(
            gw2.reshape(KT, 128, 2 * HL).transpose(1, 0, 2)
        ).astype(NPBF)
        selc = np.zeros((2 * HL, HL * 128), dtype=NPBF)
        for h in range(HL):
            selc[h, 128 * h : 128 * h + 64] = 1.0
            selc[HL + h, 128 * h + 64 : 128 * h + 128] = 1.0
        gsv = np.array([0.125] * HL + [-0.125] * HL, dtype=np.float32).reshape(2 * HL, 1)
        gbv = np.array([0.0] * HL + [0.125] * HL, dtype=np.float32).reshape(2 * HL, 1)
        gll = gate_logit[4 * hg : 4 * hg + 4]
        glog = np.ascontiguousarray(
            np.concatenate([gll, gll]).reshape(2 * HL, 1)
        ).astype(np.float32)
        in_maps.append(
            {
                "xt": xt,
                "wq": wq,
                "wk": wk,
                "wv": np.ascontiguousarray(wv),
                "wo": np.ascontiguousarray(wo),
                "wg": wg,
                "glog": glog,
                "crep": crep,
                "srep": srep,
                "maskd": maskd,
                "selc": selc,
                "gsv": gsv,
                "gbv": gbv,
            }
        )
    return in_maps


def _run(inputs, trace=False):
    global _CACHED_NC
    if _CACHED_NC is None:
        _CACHED_NC = _build_nc()
    in_maps = _host_prep(**{k: np.asarray(v) for k, v in inputs.items()})
    res = run_bass_kernel_spmd(
        _CACHED_NC, in_maps, core_ids=list(range(8)), trace=trace
    )
    y = np.zeros((B, T, D), dtype=np.float32)
    for core in range(8):
        y[core // 4] += res.results[core]["y"]
    return y, res


def kernel(**inputs) -> np.ndarray:
    y, _ = _run(inputs, trace=False)
    return y
